# revision 19
# baseline (speedup 1.0000x reference)
"""Block-sparse local+vertical-stride causal attention for Trainium2 (Bass/Tile).

Problem: B=1, S=2048, H=32, D=128, sparse_block=64, local_blocks=16,
vert_stride=8, head_sliding_step=1. Mask per head h:
  causal(q,k) AND ( (q_blk - k_blk < 16) OR ((k_blk + h + 1) % 8 == 0) )

Sharding: 8 cores; core c computes heads {c, c+8, c+16, c+24}. All four share
the same vertical-stride residue r = (7 - c) % 8, so a single compiled SPMD
program works for every core with per-core *data* (small masks + pre-gathered
vertical K/V blocks); the code is identical on all cores.

Device algorithm per (head, q-window of 256 tokens = 4 sparse blocks):
  scores S_T[k, q] on the PE in bf16 (same PE rate as fp32r, half the
  DMA/SBUF), tiled to track the sparse structure exactly:
    t >= 4:  L0 (k-blocks 4t-16,4t-15; only qb0 -> 64 q-cols)
             L1 (4t-14,4t-13; qb0..2 -> 192 q-cols)
             L2..L7 (4t-12..4t-1; full 256)
             D0 (4t,4t+1; full 256, causal triangle on chunk 0)
             D1 (4t+2,4t+3; 128 q-cols, causal triangle)
             vert (pre-gathered blocks {r, r+8}; full 256; per-partition
                   0/1 vmask selects blocks with kb < 4t-12)
    t < 4:   2t full L-tiles from block 0 (all-local) + D0 + D1
  One exp per PSUM region on ScalarE (scale folded in; bf16 out). Masking:
    - wm_small [128, 256] multiplicative mask on the L0/L1 region (dead
      sub-blocks + vert/local double-count resolution) - per-window data
    - constant aligned [128,128] causal triangle multiplied into the two
      diag chunks
    - vmask per-partition tensor_scalar on the vert tile
  PV: out_T[d, q] += V.T-stationary @ exp-moving; denominator via a
  ones-column matmul chain riding the same PSUM bank. Host divides.
Host reassembles heads and flips the per-head [d, q] layout to [q, d].
"""

import sys
import types

import numpy as np

# ----------------------------------------------------------------------------
# problem constants (hardcoded per contract; kernel.py must be self-contained)
B, S, H, D = 1, 2048, 32, 128
BLOCK = 64
LOCAL = 16
VERT = 8
NCORES = 8
HPC = H // NCORES  # heads per core (4)
QT = 256  # q tokens per window
NT = S // QT  # 8 windows per head
NKT = S // 128  # 16 k-tiles of 128 tokens per head
SCALE = float(D) ** -0.5

MM_DT = "bfloat16"  # matmul input dtype ("bfloat16" or "float32r")

# psA column layout for t >= 4: [L0 64 | L1 192 | L2..L7 6x256 | D1 128]
# = 1920 of a 4-bank (2048 col) buffer; for t < 4: [L 512t | D1 128].
# D1 always sits at the end of the L region, within a single bank.
PSA_COLS = 1792
# psB column layout: [vert 256 | D0 256] = 512 = exactly one PSUM bank,
# so bufs=2 costs 2 banks. Total: psA 4 + psB 2 + psPV 2 = 8 banks.
PSB_COLS = 512


def _install_ntff_shim():
    """bass_utils wants antenv.axon_hooks (absent in this image); provide it,
    backed by the ctypes NTFF profiler from trn_agent_boot when available."""
    if "antenv.axon_hooks" in sys.modules:
        return
    hook = None
    try:
        from trn_agent_boot.trn_boot import _ntff_profile_via_ctypes

        hook = _ntff_profile_via_ctypes("/opt/axon/libaxon_pjrt.so")
    except Exception:
        hook = None
    m = types.ModuleType("antenv.axon_hooks")
    m.get_axon_ntff_profile_hook = lambda: hook
    m.set_axon_ntff_profile_hook = lambda h: None
    sys.modules["antenv.axon_hooks"] = m


_NC = None


def _build():
    """Build + compile the per-core Bass program (one NEFF, all cores)."""
    global _NC
    if _NC is not None:
        return _NC
    import concourse.mybir as mybir
    import concourse.tile as tile
    from concourse import bacc

    f32 = mybir.dt.float32
    mdt = getattr(mybir.dt, MM_DT)

    nc = bacc.Bacc("TRN2", target_bir_lowering=False, debug=False,
                   num_devices=NCORES)

    qt_d = nc.dram_tensor("qt", [HPC, D, S], mdt, kind="ExternalInput")
    kt_d = nc.dram_tensor("kt", [HPC, D, S], mdt, kind="ExternalInput")
    v_d = nc.dram_tensor("v", [HPC, S, D], mdt, kind="ExternalInput")
    ktv_d = nc.dram_tensor("ktv", [HPC, D, 128], mdt, kind="ExternalInput")
    vv_d = nc.dram_tensor("vv", [HPC, 128, D], mdt, kind="ExternalInput")
    wm_d = nc.dram_tensor("wm", [NT - 4, 128, 256], mdt, kind="ExternalInput")
    tri_d = nc.dram_tensor("tri", [128, 128], mdt, kind="ExternalInput")
    vm_d = nc.dram_tensor("vmask", [NT, 128], f32, kind="ExternalInput")
    o_d = nc.dram_tensor("o", [HPC, D, S], f32, kind="ExternalOutput")
    den_d = nc.dram_tensor("den", [HPC, 1, S], f32, kind="ExternalOutput")

    with tile.TileContext(nc) as tc:
        with (
            tc.tile_pool(name="consts", bufs=1) as consts,
            tc.tile_pool(name="io", bufs=2) as io,
            tc.tile_pool(name="exps", bufs=2) as exps,
            tc.tile_pool(name="psA", bufs=1, space="PSUM") as psA,
            tc.tile_pool(name="psB", bufs=2, space="PSUM") as psB,
            tc.tile_pool(name="psPV", bufs=2, space="PSUM") as psPV,
        ):
            tri = consts.tile([128, 128], mdt)
            nc.gpsimd.dma_start(out=tri, in_=tri_d.ap())
            vmask = consts.tile([128, NT], f32)
            nc.gpsimd.dma_start(out=vmask, in_=vm_d.ap().rearrange("t p -> p t"))
            wmask = consts.tile([128, NT - 4, 256], mdt)
            nc.gpsimd.dma_start(out=wmask,
                                in_=wm_d.ap().rearrange("t p q -> p t q"))
            ones_f32 = consts.tile([128, 1], f32)
            nc.vector.memset(ones_f32, 1.0)
            ones_col = consts.tile([128, 1], mdt)
            nc.vector.tensor_copy(out=ones_col, in_=ones_f32)

            def load_head(h, fine):
                """Allocate per-head io tiles and issue their input DMAs.
                fine=True (head 0): window-granular chunks alternating
                between the two HWDGE queues (sync/scalar) so the PE can
                start as soon as the first 128KB lands."""
                qt_t = io.tile([128, S], mdt, tag="qt")
                kt_t = io.tile([128, S], mdt, tag="kt")
                v_t = io.tile([128, NKT, 128], mdt, tag="v")
                ktv_t = io.tile([128, 128], mdt, tag="ktv")
                vv_t = io.tile([128, 128], mdt, tag="vv")
                t = {"qt": qt_t, "kt": kt_t, "v": v_t, "ktv": ktv_t,
                     "vv": vv_t}
                vre = v_d.ap()[h].rearrange("(j p) d -> p j d", p=128)
                if fine:
                    # chunk 0 rides the gpsimd (SWDGE) queue, which starts
                    # several us before the HWDGE queues finish their
                    # preamble; later chunks alternate sync/scalar HWDGE
                    eng = [nc.gpsimd, nc.sync, nc.scalar, nc.sync, nc.scalar]
                    # (kt cols, qt cols, v k-tiles) per chunk, matched to
                    # window consumption order
                    chunks = [(256, 256, 2), (512, 512, 4), (1024, 1024, 8),
                              (1536, 1536, 12), (2048, 2048, 16)]
                    pk = pq = pv = 0
                    for i, (ck, cq, cv) in enumerate(chunks):
                        e = eng[i]
                        e.dma_start(out=t["kt"][:, pk:ck],
                                    in_=kt_d.ap()[h][:, pk:ck])
                        e.dma_start(out=t["qt"][:, pq:cq],
                                    in_=qt_d.ap()[h][:, pq:cq])
                        e.dma_start(out=t["v"][:, pv:cv, :],
                                    in_=vre[:, pv:cv, :])
                        pk, pq, pv = ck, cq, cv
                else:
                    nc.sync.dma_start(out=t["kt"][:, 0:512],
                                      in_=kt_d.ap()[h][:, 0:512])
                    nc.sync.dma_start(out=t["qt"][:, 0:512],
                                      in_=qt_d.ap()[h][:, 0:512])
                    nc.sync.dma_start(out=t["v"][:, 0:4, :], in_=vre[:, 0:4, :])
                    nc.sync.dma_start(out=t["kt"][:, 512:2048],
                                      in_=kt_d.ap()[h][:, 512:2048])
                    nc.sync.dma_start(out=t["qt"][:, 512:2048],
                                      in_=qt_d.ap()[h][:, 512:2048])
                    nc.sync.dma_start(out=t["v"][:, 4:, :], in_=vre[:, 4:, :])
                nc.gpsimd.dma_start(out=t["ktv"], in_=ktv_d.ap()[h])
                nc.gpsimd.dma_start(out=t["vv"], in_=vv_d.ap()[h])
                return t

            def make_ctx(h, fine):
                c = load_head(h, fine)
                c["h"] = h
                outT_t = io.tile([128, S], f32, tag="outT")
                den_t = io.tile([1, S], f32, tag="den_sb")
                c["outT"] = outT_t
                c["den"] = den_t
                return c

            def emit_scores(c, t):
                """Scores matmuls + exp + masks for window t. Returns the
                (exp-slice, v-slice) list for the PV/den stage."""
                kt_sb, v_sb = c["kt"], c["v"]
                q_sl = c["qt"][:, t * QT:(t + 1) * QT]
                srcs = []

                vert = t >= 4
                acols = PSA_COLS if t >= 4 else 512 * t
                sA = psA.tile([128, acols + 128], f32, tag="sA")
                if t >= 4:
                    # L0: k-blocks 4t-16,4t-15 (k-tile 2t-8), qb0 only
                    nc.tensor.matmul(
                        sA[:, 0:64],
                        kt_sb[:, (2 * t - 8) * 128:(2 * t - 7) * 128],
                        q_sl[:, 0:64], start=True, stop=True,
                    )
                    # L1: k-blocks 4t-14,4t-13 (k-tile 2t-7), qb0..2
                    nc.tensor.matmul(
                        sA[:, 64:256],
                        kt_sb[:, (2 * t - 7) * 128:(2 * t - 6) * 128],
                        q_sl[:, 0:192], start=True, stop=True,
                    )
                    # L2..L7: k-tiles 2t-6 .. 2t-1, full 256
                    for j in range(6):
                        kt_i = 2 * t - 6 + j
                        nc.tensor.matmul(
                            sA[:, 256 * (j + 1):256 * (j + 2)],
                            kt_sb[:, kt_i * 128:(kt_i + 1) * 128],
                            q_sl, start=True, stop=True,
                        )
                else:
                    for j in range(2 * t):
                        nc.tensor.matmul(
                            sA[:, 256 * j:256 * (j + 1)],
                            kt_sb[:, j * 128:(j + 1) * 128],
                            q_sl, start=True, stop=True,
                        )
                # D1 (k-tile 2t+1, q chunk 1) rides at the end of the A
                # region so its exp is fused into the expA activation
                nc.tensor.matmul(
                    sA[:, acols:acols + 128],
                    kt_sb[:, (2 * t + 1) * 128:(2 * t + 2) * 128],
                    q_sl[:, 128:256], start=True, stop=True,
                )
                expA = exps.tile([128, acols + 128], mdt, tag="expA")
                nc.scalar.activation(
                    expA, sA, mybir.ActivationFunctionType.Exp, scale=SCALE,
                )
                if t >= 4:
                    # data-driven mask on the L0/L1 region (dead blocks,
                    # vert/local double-count)
                    nc.vector.tensor_mul(
                        expA[:, 0:256], expA[:, 0:256], wmask[:, t - 4, :]
                    )
                # causal triangle on D1
                nc.vector.tensor_mul(
                    expA[:, acols:acols + 128], expA[:, acols:acols + 128],
                    tri,
                )

                # B region: [vert 0:256 | D0 256:512]
                sB = psB.tile([128, PSB_COLS], f32, tag="sB")
                nc.tensor.matmul(
                    sB[:, 256:512],
                    kt_sb[:, (2 * t) * 128:(2 * t + 1) * 128],
                    q_sl, start=True, stop=True,
                )
                if vert:
                    nc.tensor.matmul(
                        sB[:, 0:256], c["ktv"], q_sl,
                        start=True, stop=True,
                    )
                expB = exps.tile([128, PSB_COLS], mdt, tag="expB")
                lo = 0 if vert else 256
                nc.scalar.activation(
                    expB[:, lo:PSB_COLS], sB[:, lo:PSB_COLS],
                    mybir.ActivationFunctionType.Exp, scale=SCALE,
                )
                # causal triangle on D0 chunk 0
                nc.vector.tensor_mul(expB[:, 256:384], expB[:, 256:384], tri)
                if vert:
                    nc.vector.tensor_scalar_mul(
                        out=expB[:, 0:256], in0=expB[:, 0:256],
                        scalar1=vmask[:, t:t + 1],
                    )
                # srcs ordered so the first entry is full-width: the first
                # matmul of the PV/den chains must cover the whole [0:256]
                # q range so has_written is set everywhere
                if t >= 4:
                    for j in range(6):
                        srcs.append((expA[:, 256 * (j + 1):256 * (j + 2)],
                                     v_sb[:, 2 * t - 6 + j, :]))
                else:
                    for j in range(2 * t):
                        srcs.append((expA[:, 256 * j:256 * (j + 1)],
                                     v_sb[:, j, :]))
                srcs.append((expB[:, 256:512], v_sb[:, 2 * t, :]))
                if vert:
                    srcs.append((expB[:, 0:256], c["vv"]))
                    srcs.append((expA[:, 0:64], v_sb[:, 2 * t - 8, :]))
                    srcs.append((expA[:, 64:256], v_sb[:, 2 * t - 7, :]))
                srcs.append((expA[:, acols:acols + 128],
                             v_sb[:, 2 * t + 1, :]))
                return srcs

            def emit_pv(c, t, srcs):
                """PV + denominator matmul chains, then stage out_T and
                den to SBUF and DMA out_T."""
                pv = psPV.tile([128, 512], f32, tag="pv")
                n_mm = len(srcs)
                # q-column range of each exp slice within the window:
                # L0 -> [0:64], L1 -> [0:192], D1 -> [128:256], else full
                def qrange(e_sl):
                    n = e_sl.shape[-1]
                    if n == 64:
                        return 0, 64
                    if n == 192:
                        return 0, 192
                    if n == 128:
                        return 128, 256
                    return 0, 256

                for k, (e_sl, v_sl) in enumerate(srcs):
                    lo, hi = qrange(e_sl)
                    nc.tensor.matmul(
                        pv[:, lo:hi], v_sl, e_sl,
                        start=(k == 0), stop=(k == n_mm - 1),
                    )
                for k, (e_sl, _) in enumerate(srcs):
                    lo, hi = qrange(e_sl)
                    nc.tensor.matmul(
                        pv[0:1, 256 + lo:256 + hi], ones_col, e_sl,
                        start=(k == 0), stop=(k == n_mm - 1),
                    )
                nc.vector.tensor_copy(
                    out=c["den"][0:1, t * QT:(t + 1) * QT],
                    in_=pv[0:1, 256:512],
                )
                nc.vector.tensor_copy(
                    out=c["outT"][:, t * QT:(t + 1) * QT], in_=pv[:, 0:256]
                )
                nc.sync.dma_start(
                    out=o_d.ap()[c["h"]][:, t * QT:(t + 1) * QT],
                    in_=c["outT"][:, t * QT:(t + 1) * QT],
                )
                # den halves DMA'd as they complete to shorten the tail
                if t == 3:
                    nc.sync.dma_start(out=den_d.ap()[c["h"]][:, 0:1024],
                                      in_=c["den"][:, 0:1024])
                elif t == NT - 1:
                    nc.sync.dma_start(out=den_d.ap()[c["h"]][:, 1024:2048],
                                      in_=c["den"][:, 1024:2048])

            # software-pipelined emission ACROSS heads: scores(w+1) lands
            # before pv(w) so the PE always has matmul work while ACT/DVE
            # process window w, including at head boundaries
            ctxs = {0: make_ctx(0, fine=True)}
            prev = None
            for h in range(HPC):
                for t in range(NT):
                    srcs = emit_scores(ctxs[h], t)
                    if prev is not None:
                        emit_pv(ctxs[prev[0]], prev[1], prev[2])
                    if t == 4 and h + 1 < HPC:
                        # prefetch the next head's inputs while this head
                        # still has ~4 windows of compute left
                        ctxs[h + 1] = make_ctx(h + 1, fine=False)
                    prev = (h, t, srcs)
            emit_pv(ctxs[prev[0]], prev[1], prev[2])

    nc.compile()
    _NC = nc
    return nc


def _host_masks(r):
    """Per-core mask data: wm [NT,128,256], vmask [NT,128], tri [128,128]."""
    # wm: multiplies expA[:, 0:256] (L0 cols 0:64 + L1 cols 64:256), t >= 4.
    # partition p: L0 -> block 4t-16 (p<64) / 4t-15 (p>=64)
    #              L1 -> block 4t-14 (p<64) / 4t-13 (p>=64)
    wm_full = np.ones((NT, 128, 256), dtype=np.float32)
    wm = wm_full  # filled below; rows [4:] shipped to the device
    for t in range(4, NT):
        bl0, bl1 = 4 * t - 16, 4 * t - 15
        bl2, bl3 = 4 * t - 14, 4 * t - 13
        vc = lambda kb: kb % VERT == r  # vert tile covers kb (kb < 4t-12 here)
        # L0 region (cols 0:64 = qb0)
        wm[t, 0:64, 0:64] = 0.0  # block 4t-16 never local
        if vc(bl1):
            wm[t, 64:128, 0:64] = 0.0  # vert tile owns block 4t-15
        # L1 region (cols 64:256 = qb0..2)
        if vc(bl2):
            wm[t, 0:64, 64:256] = 0.0
        else:
            wm[t, 0:64, 64 + 128:256] = 0.0  # qb2 not local for 4t-14
        if vc(bl3):
            wm[t, 64:128, 64:256] = 0.0
        # block 4t-13 local for qb0..2 (all L1 cols) when not vert-covered
    # vmask: vert tile partitions p -> kb = r (p<64) / r+8 (p>=64);
    # active iff kb < 4t-12
    vm = np.zeros((NT, 128), dtype=np.float32)
    for t in range(4, NT):
        if r < 4 * t - 12:
            vm[t, 0:64] = 1.0
        if r + 8 < 4 * t - 12:
            vm[t, 64:128] = 1.0
    tri = (np.arange(128)[None, :] >= np.arange(128)[:, None]).astype(
        np.float32
    )
    return np.ascontiguousarray(wm_full[4:]), vm, tri


def _host_prep(query, key, value, core):
    """Per-core input dict. query/key/value: [B, S, H, D] float32 (full)."""
    import ml_dtypes

    heads = [core + NCORES * i for i in range(HPC)]
    r = (7 - core) % VERT
    q = query[0][:, heads, :]  # [S, 4, D]
    k = key[0][:, heads, :]
    v = value[0][:, heads, :]
    qt = np.ascontiguousarray(q.transpose(1, 2, 0))  # [4, D, S]
    kt = np.ascontiguousarray(k.transpose(1, 2, 0))
    vn = np.ascontiguousarray(v.transpose(1, 0, 2))  # [4, S, D]
    # vertical gather: k-tokens of blocks {r, r+8}
    vtok = np.concatenate([
        np.arange(r * BLOCK, (r + 1) * BLOCK),
        np.arange((r + 8) * BLOCK, (r + 9) * BLOCK),
    ])
    ktv = np.ascontiguousarray(kt[:, :, vtok])  # [4, D, 128]
    vv = np.ascontiguousarray(vn[:, vtok, :])  # [4, 128, D]

    wm, vm, tri = _host_masks(r)

    if MM_DT == "bfloat16":
        dt = ml_dtypes.bfloat16
    else:
        dt = np.float32
    return {
        "qt": qt.astype(dt), "kt": kt.astype(dt), "v": vn.astype(dt),
        "ktv": ktv.astype(dt), "vv": vv.astype(dt),
        "wm": wm.astype(dt), "tri": tri.astype(dt), "vmask": vm,
    }


def kernel(query, key, value, _trace=False, _tmpdir=None):
    """Full-input entry point: [1, 2048, 32, 128] f32 each -> same shape."""
    _install_ntff_shim()
    from concourse.bass_utils import run_bass_kernel_spmd

    query = np.asarray(query, dtype=np.float32)
    key = np.asarray(key, dtype=np.float32)
    value = np.asarray(value, dtype=np.float32)

    nc = _build()
    in_maps = [_host_prep(query, key, value, c) for c in range(NCORES)]
    res = run_bass_kernel_spmd(
        nc, in_maps, core_ids=list(range(NCORES)),
        trace=_trace, tmpdir=_tmpdir,
    )
    out = np.empty((B, S, H, D), dtype=np.float32)
    for c in range(NCORES):
        o = res.results[c]["o"] / res.results[c]["den"]  # [4, D, S] / [4, 1, S]
        for i in range(HPC):
            out[0, :, c + NCORES * i, :] = o[i].T
    kernel.last_result = res
    return out


# revision 23
# speedup vs baseline: 1.0480x; 1.0480x over previous
"""Block-sparse local+vertical-stride causal attention for Trainium2 (Bass/Tile).

Problem: B=1, S=2048, H=32, D=128, sparse_block=64, local_blocks=16,
vert_stride=8, head_sliding_step=1. Mask per head h:
  causal(q,k) AND ( (q_blk - k_blk < 16) OR ((k_blk + h + 1) % 8 == 0) )

Sharding: 8 cores; core c computes heads {c, c+8, c+16, c+24}. All four share
the same vertical-stride residue r = (7 - c) % 8, so a single compiled SPMD
program works for every core with per-core *data* (small masks + pre-gathered
vertical K/V blocks); the code is identical on all cores.

Device algorithm per (head, q-window of 256 tokens = 4 sparse blocks):
  scores S_T[k, q] on the PE in bf16 (same PE rate as fp32r, half the
  DMA/SBUF), tiled to track the sparse structure exactly:
    t >= 4:  L0 (k-blocks 4t-16,4t-15; only qb0 -> 64 q-cols)
             L1 (4t-14,4t-13; qb0..2 -> 192 q-cols)
             L2..L7 (4t-12..4t-1; full 256)
             D0 (4t,4t+1; full 256, causal triangle on chunk 0)
             D1 (4t+2,4t+3; 128 q-cols, causal triangle)
             vert (pre-gathered blocks {r, r+8}; full 256; per-partition
                   0/1 vmask selects blocks with kb < 4t-12)
    t < 4:   2t full L-tiles from block 0 (all-local) + D0 + D1
  One exp per PSUM region on ScalarE (scale folded in; bf16 out). Masking:
    - wm_small [128, 256] multiplicative mask on the L0/L1 region (dead
      sub-blocks + vert/local double-count resolution) - per-window data
    - constant aligned [128,128] causal triangle multiplied into the two
      diag chunks
    - vmask per-partition tensor_scalar on the vert tile
  PV: out_T[d, q] += V.T-stationary @ exp-moving; denominator via a
  ones-column matmul chain riding the same PSUM bank. Host divides.
Host reassembles heads and flips the per-head [d, q] layout to [q, d].
"""

import sys
import types

import numpy as np

# ----------------------------------------------------------------------------
# problem constants (hardcoded per contract; kernel.py must be self-contained)
B, S, H, D = 1, 2048, 32, 128
BLOCK = 64
LOCAL = 16
VERT = 8
NCORES = 8
HPC = H // NCORES  # heads per core (4)
QT = 256  # q tokens per window
NT = S // QT  # 8 windows per head
NKT = S // 128  # 16 k-tiles of 128 tokens per head
SCALE = float(D) ** -0.5

MM_DT = "bfloat16"  # matmul input dtype ("bfloat16" or "float32r")

# psA column layout for t >= 4: [L0 64 | L1 192 | L2..L7 6x256 | D1 128]
# = 1920 of a 4-bank (2048 col) buffer; for t < 4: [L 512t | D1 128].
# D1 always sits at the end of the L region, within a single bank.
PSA_COLS = 1792
# psB column layout: [vert 256 | D0 256] = 512 = exactly one PSUM bank,
# so bufs=2 costs 2 banks. Total: psA 4 + psB 2 + psPV 2 = 8 banks.
PSB_COLS = 512


def _install_ntff_shim():
    """bass_utils wants antenv.axon_hooks (absent in this image); provide it,
    backed by the ctypes NTFF profiler from trn_agent_boot when available."""
    if "antenv.axon_hooks" in sys.modules:
        return
    hook = None
    try:
        from trn_agent_boot.trn_boot import _ntff_profile_via_ctypes

        hook = _ntff_profile_via_ctypes("/opt/axon/libaxon_pjrt.so")
    except Exception:
        hook = None
    m = types.ModuleType("antenv.axon_hooks")
    m.get_axon_ntff_profile_hook = lambda: hook
    m.set_axon_ntff_profile_hook = lambda h: None
    sys.modules["antenv.axon_hooks"] = m


_NC = None


def _build():
    """Build + compile the per-core Bass program (one NEFF, all cores)."""
    global _NC
    if _NC is not None:
        return _NC
    import concourse.mybir as mybir
    import concourse.tile as tile
    from concourse import bacc

    f32 = mybir.dt.float32
    mdt = getattr(mybir.dt, MM_DT)

    nc = bacc.Bacc("TRN2", target_bir_lowering=False, debug=False,
                   num_devices=NCORES)

    qt_d = nc.dram_tensor("qt", [HPC, D, S], mdt, kind="ExternalInput")
    kt_d = nc.dram_tensor("kt", [HPC, D, S], mdt, kind="ExternalInput")
    v_d = nc.dram_tensor("v", [HPC, S, D], mdt, kind="ExternalInput")
    ktv_d = nc.dram_tensor("ktv", [HPC, D, 128], mdt, kind="ExternalInput")
    vv_d = nc.dram_tensor("vv", [HPC, 128, D], mdt, kind="ExternalInput")
    wm_d = nc.dram_tensor("wm", [NT - 4, 128, 256], mdt, kind="ExternalInput")
    tri_d = nc.dram_tensor("tri", [128, 128], mdt, kind="ExternalInput")
    vm_d = nc.dram_tensor("vmask", [NT, 128], f32, kind="ExternalInput")
    o_d = nc.dram_tensor("o", [HPC, D, S], f32, kind="ExternalOutput")
    den_d = nc.dram_tensor("den", [HPC, 1, S], f32, kind="ExternalOutput")

    with tile.TileContext(nc) as tc:
        with (
            tc.tile_pool(name="consts", bufs=1) as consts,
            tc.tile_pool(name="io", bufs=2) as io,
            tc.tile_pool(name="exps", bufs=2) as exps,
            tc.tile_pool(name="psA", bufs=1, space="PSUM") as psA,
            tc.tile_pool(name="psB", bufs=2, space="PSUM") as psB,
            tc.tile_pool(name="psPV", bufs=2, space="PSUM") as psPV,
        ):
            tri = consts.tile([128, 128], mdt)
            nc.gpsimd.dma_start(out=tri, in_=tri_d.ap())
            vmask = consts.tile([128, NT], f32)
            nc.gpsimd.dma_start(out=vmask, in_=vm_d.ap().rearrange("t p -> p t"))
            wmask = consts.tile([128, NT - 4, 256], mdt)
            ones_f32 = consts.tile([128, 1], f32)
            nc.vector.memset(ones_f32, 1.0)
            ones_col = consts.tile([128, 1], mdt)
            nc.vector.tensor_copy(out=ones_col, in_=ones_f32)

            def load_head(h, fine):
                """Allocate per-head io tiles and issue their input DMAs.
                fine=True (head 0): window-granular chunks alternating
                between the two HWDGE queues (sync/scalar) so the PE can
                start as soon as the first 128KB lands."""
                qt_t = io.tile([128, S], mdt, tag="qt")
                kt_t = io.tile([128, S], mdt, tag="kt")
                v_t = io.tile([128, NKT, 128], mdt, tag="v")
                ktv_t = io.tile([128, 128], mdt, tag="ktv")
                vv_t = io.tile([128, 128], mdt, tag="vv")
                t = {"qt": qt_t, "kt": kt_t, "v": v_t, "ktv": ktv_t,
                     "vv": vv_t}
                vre = v_d.ap()[h].rearrange("(j p) d -> p j d", p=128)
                if fine:
                    # minimal head-of-queue chunk (window 0's kt+qt only),
                    # then window-matched chunks alternating the two HWDGE
                    # queues (sync/scalar)
                    nc.sync.dma_start(out=t["kt"][:, 0:256],
                                      in_=kt_d.ap()[h][:, 0:256])
                    nc.sync.dma_start(out=t["qt"][:, 0:256],
                                      in_=qt_d.ap()[h][:, 0:256])
                    nc.scalar.dma_start(out=t["v"][:, 0:2, :],
                                        in_=vre[:, 0:2, :])
                    eng = [nc.scalar, nc.sync, nc.scalar, nc.sync]
                    chunks = [(512, 512, 4), (1024, 1024, 8),
                              (1536, 1536, 12), (2048, 2048, 16)]
                    pk, pq, pv = 256, 256, 2
                    for i, (ck, cq, cv) in enumerate(chunks):
                        e = eng[i]
                        e.dma_start(out=t["kt"][:, pk:ck],
                                    in_=kt_d.ap()[h][:, pk:ck])
                        e.dma_start(out=t["qt"][:, pq:cq],
                                    in_=qt_d.ap()[h][:, pq:cq])
                        e.dma_start(out=t["v"][:, pv:cv, :],
                                    in_=vre[:, pv:cv, :])
                        pk, pq, pv = ck, cq, cv
                else:
                    nc.sync.dma_start(out=t["kt"][:, 0:512],
                                      in_=kt_d.ap()[h][:, 0:512])
                    nc.sync.dma_start(out=t["qt"][:, 0:512],
                                      in_=qt_d.ap()[h][:, 0:512])
                    nc.sync.dma_start(out=t["v"][:, 0:4, :], in_=vre[:, 0:4, :])
                    nc.sync.dma_start(out=t["kt"][:, 512:2048],
                                      in_=kt_d.ap()[h][:, 512:2048])
                    nc.sync.dma_start(out=t["qt"][:, 512:2048],
                                      in_=qt_d.ap()[h][:, 512:2048])
                    nc.sync.dma_start(out=t["v"][:, 4:, :], in_=vre[:, 4:, :])
                nc.gpsimd.dma_start(out=t["ktv"], in_=ktv_d.ap()[h])
                nc.gpsimd.dma_start(out=t["vv"], in_=vv_d.ap()[h])
                return t

            def make_ctx(h, fine):
                c = load_head(h, fine)
                c["h"] = h
                outT_t = io.tile([128, S], f32, tag="outT")
                den_t = io.tile([1, S], f32, tag="den_sb")
                c["outT"] = outT_t
                c["den"] = den_t
                return c

            def emit_scores(c, t):
                """Scores matmuls + exp + masks for window t. Returns the
                (exp-slice, v-slice) list for the PV/den stage."""
                kt_sb, v_sb = c["kt"], c["v"]
                q_sl = c["qt"][:, t * QT:(t + 1) * QT]
                srcs = []

                vert = t >= 4
                acols = PSA_COLS if t >= 4 else 512 * t
                sA = psA.tile([128, acols + 128], f32, tag="sA")
                if t >= 4:
                    # L0: k-blocks 4t-16,4t-15 (k-tile 2t-8), qb0 only
                    nc.tensor.matmul(
                        sA[:, 0:64],
                        kt_sb[:, (2 * t - 8) * 128:(2 * t - 7) * 128],
                        q_sl[:, 0:64], start=True, stop=True,
                    )
                    # L1: k-blocks 4t-14,4t-13 (k-tile 2t-7), qb0..2
                    nc.tensor.matmul(
                        sA[:, 64:256],
                        kt_sb[:, (2 * t - 7) * 128:(2 * t - 6) * 128],
                        q_sl[:, 0:192], start=True, stop=True,
                    )
                    # L2..L7: k-tiles 2t-6 .. 2t-1, full 256
                    for j in range(6):
                        kt_i = 2 * t - 6 + j
                        nc.tensor.matmul(
                            sA[:, 256 * (j + 1):256 * (j + 2)],
                            kt_sb[:, kt_i * 128:(kt_i + 1) * 128],
                            q_sl, start=True, stop=True,
                        )
                else:
                    for j in range(2 * t):
                        nc.tensor.matmul(
                            sA[:, 256 * j:256 * (j + 1)],
                            kt_sb[:, j * 128:(j + 1) * 128],
                            q_sl, start=True, stop=True,
                        )
                # D1 (k-tile 2t+1, q chunk 1) rides at the end of the A
                # region so its exp is fused into the expA activation
                nc.tensor.matmul(
                    sA[:, acols:acols + 128],
                    kt_sb[:, (2 * t + 1) * 128:(2 * t + 2) * 128],
                    q_sl[:, 128:256], start=True, stop=True,
                )
                expA = exps.tile([128, acols + 128], mdt, tag="expA")
                nc.scalar.activation(
                    expA, sA, mybir.ActivationFunctionType.Exp, scale=SCALE,
                )
                if t >= 4:
                    # data-driven mask on the L0/L1 region (dead blocks,
                    # vert/local double-count)
                    nc.vector.tensor_mul(
                        expA[:, 0:256], expA[:, 0:256], wmask[:, t - 4, :]
                    )
                # causal triangle on D1
                nc.vector.tensor_mul(
                    expA[:, acols:acols + 128], expA[:, acols:acols + 128],
                    tri,
                )

                # B region: [vert 0:256 | D0 256:512]
                sB = psB.tile([128, PSB_COLS], f32, tag="sB")
                nc.tensor.matmul(
                    sB[:, 256:512],
                    kt_sb[:, (2 * t) * 128:(2 * t + 1) * 128],
                    q_sl, start=True, stop=True,
                )
                if vert:
                    nc.tensor.matmul(
                        sB[:, 0:256], c["ktv"], q_sl,
                        start=True, stop=True,
                    )
                expB = exps.tile([128, PSB_COLS], mdt, tag="expB")
                lo = 0 if vert else 256
                nc.scalar.activation(
                    expB[:, lo:PSB_COLS], sB[:, lo:PSB_COLS],
                    mybir.ActivationFunctionType.Exp, scale=SCALE,
                )
                # causal triangle on D0 chunk 0
                nc.vector.tensor_mul(expB[:, 256:384], expB[:, 256:384], tri)
                if vert:
                    nc.vector.tensor_scalar_mul(
                        out=expB[:, 0:256], in0=expB[:, 0:256],
                        scalar1=vmask[:, t:t + 1],
                    )
                # srcs ordered so the first entry is full-width: the first
                # matmul of the PV/den chains must cover the whole [0:256]
                # q range so has_written is set everywhere
                if t >= 4:
                    for j in range(6):
                        srcs.append((expA[:, 256 * (j + 1):256 * (j + 2)],
                                     v_sb[:, 2 * t - 6 + j, :]))
                else:
                    for j in range(2 * t):
                        srcs.append((expA[:, 256 * j:256 * (j + 1)],
                                     v_sb[:, j, :]))
                srcs.append((expB[:, 256:512], v_sb[:, 2 * t, :]))
                if vert:
                    srcs.append((expB[:, 0:256], c["vv"]))
                    srcs.append((expA[:, 0:64], v_sb[:, 2 * t - 8, :]))
                    srcs.append((expA[:, 64:256], v_sb[:, 2 * t - 7, :]))
                srcs.append((expA[:, acols:acols + 128],
                             v_sb[:, 2 * t + 1, :]))
                return srcs

            def emit_pv(c, t, srcs):
                """PV + denominator matmul chains, then stage out_T and
                den to SBUF and DMA out_T."""
                pv = psPV.tile([128, 512], f32, tag="pv")
                n_mm = len(srcs)
                # q-column range of each exp slice within the window:
                # L0 -> [0:64], L1 -> [0:192], D1 -> [128:256], else full
                def qrange(e_sl):
                    n = e_sl.shape[-1]
                    if n == 64:
                        return 0, 64
                    if n == 192:
                        return 0, 192
                    if n == 128:
                        return 128, 256
                    return 0, 256

                for k, (e_sl, v_sl) in enumerate(srcs):
                    lo, hi = qrange(e_sl)
                    nc.tensor.matmul(
                        pv[:, lo:hi], v_sl, e_sl,
                        start=(k == 0), stop=(k == n_mm - 1),
                    )
                for k, (e_sl, _) in enumerate(srcs):
                    lo, hi = qrange(e_sl)
                    nc.tensor.matmul(
                        pv[0:1, 256 + lo:256 + hi], ones_col, e_sl,
                        start=(k == 0), stop=(k == n_mm - 1),
                    )
                nc.vector.tensor_copy(
                    out=c["den"][0:1, t * QT:(t + 1) * QT],
                    in_=pv[0:1, 256:512],
                )
                nc.vector.tensor_copy(
                    out=c["outT"][:, t * QT:(t + 1) * QT], in_=pv[:, 0:256]
                )
                nc.sync.dma_start(
                    out=o_d.ap()[c["h"]][:, t * QT:(t + 1) * QT],
                    in_=c["outT"][:, t * QT:(t + 1) * QT],
                )
                # den halves DMA'd as they complete to shorten the tail
                if t == 3:
                    nc.sync.dma_start(out=den_d.ap()[c["h"]][:, 0:1024],
                                      in_=c["den"][:, 0:1024])
                elif t == NT - 1:
                    nc.sync.dma_start(out=den_d.ap()[c["h"]][:, 1024:2048],
                                      in_=c["den"][:, 1024:2048])

            # software-pipelined emission ACROSS heads: scores(w+1) lands
            # before pv(w) so the PE always has matmul work while ACT/DVE
            # process window w, including at head boundaries
            ctxs = {0: make_ctx(0, fine=True)}
            # big const (256KB, SWDGE) issued after head 0's inputs; first
            # used at window t=4 (~13us in)
            nc.gpsimd.dma_start(out=wmask,
                                in_=wm_d.ap().rearrange("t p q -> p t q"))
            prev = None
            for h in range(HPC):
                for t in range(NT):
                    srcs = emit_scores(ctxs[h], t)
                    if prev is not None:
                        emit_pv(ctxs[prev[0]], prev[1], prev[2])
                    if t == 4 and h + 1 < HPC:
                        # prefetch the next head's inputs while this head
                        # still has ~4 windows of compute left
                        ctxs[h + 1] = make_ctx(h + 1, fine=False)
                    prev = (h, t, srcs)
            emit_pv(ctxs[prev[0]], prev[1], prev[2])

    nc.compile()
    _NC = nc
    return nc


def _host_masks(r):
    """Per-core mask data: wm [NT,128,256], vmask [NT,128], tri [128,128]."""
    # wm: multiplies expA[:, 0:256] (L0 cols 0:64 + L1 cols 64:256), t >= 4.
    # partition p: L0 -> block 4t-16 (p<64) / 4t-15 (p>=64)
    #              L1 -> block 4t-14 (p<64) / 4t-13 (p>=64)
    wm_full = np.ones((NT, 128, 256), dtype=np.float32)
    wm = wm_full  # filled below; rows [4:] shipped to the device
    for t in range(4, NT):
        bl0, bl1 = 4 * t - 16, 4 * t - 15
        bl2, bl3 = 4 * t - 14, 4 * t - 13
        vc = lambda kb: kb % VERT == r  # vert tile covers kb (kb < 4t-12 here)
        # L0 region (cols 0:64 = qb0)
        wm[t, 0:64, 0:64] = 0.0  # block 4t-16 never local
        if vc(bl1):
            wm[t, 64:128, 0:64] = 0.0  # vert tile owns block 4t-15
        # L1 region (cols 64:256 = qb0..2)
        if vc(bl2):
            wm[t, 0:64, 64:256] = 0.0
        else:
            wm[t, 0:64, 64 + 128:256] = 0.0  # qb2 not local for 4t-14
        if vc(bl3):
            wm[t, 64:128, 64:256] = 0.0
        # block 4t-13 local for qb0..2 (all L1 cols) when not vert-covered
    # vmask: vert tile partitions p -> kb = r (p<64) / r+8 (p>=64);
    # active iff kb < 4t-12
    vm = np.zeros((NT, 128), dtype=np.float32)
    for t in range(4, NT):
        if r < 4 * t - 12:
            vm[t, 0:64] = 1.0
        if r + 8 < 4 * t - 12:
            vm[t, 64:128] = 1.0
    tri = (np.arange(128)[None, :] >= np.arange(128)[:, None]).astype(
        np.float32
    )
    return np.ascontiguousarray(wm_full[4:]), vm, tri


def _host_prep(query, key, value, core):
    """Per-core input dict. query/key/value: [B, S, H, D] float32 (full)."""
    import ml_dtypes

    heads = [core + NCORES * i for i in range(HPC)]
    r = (7 - core) % VERT
    q = query[0][:, heads, :]  # [S, 4, D]
    k = key[0][:, heads, :]
    v = value[0][:, heads, :]
    qt = np.ascontiguousarray(q.transpose(1, 2, 0))  # [4, D, S]
    kt = np.ascontiguousarray(k.transpose(1, 2, 0))
    vn = np.ascontiguousarray(v.transpose(1, 0, 2))  # [4, S, D]
    # vertical gather: k-tokens of blocks {r, r+8}
    vtok = np.concatenate([
        np.arange(r * BLOCK, (r + 1) * BLOCK),
        np.arange((r + 8) * BLOCK, (r + 9) * BLOCK),
    ])
    ktv = np.ascontiguousarray(kt[:, :, vtok])  # [4, D, 128]
    vv = np.ascontiguousarray(vn[:, vtok, :])  # [4, 128, D]

    wm, vm, tri = _host_masks(r)

    if MM_DT == "bfloat16":
        dt = ml_dtypes.bfloat16
    else:
        dt = np.float32
    return {
        "qt": qt.astype(dt), "kt": kt.astype(dt), "v": vn.astype(dt),
        "ktv": ktv.astype(dt), "vv": vv.astype(dt),
        "wm": wm.astype(dt), "tri": tri.astype(dt), "vmask": vm,
    }


def kernel(query, key, value, _trace=False, _tmpdir=None):
    """Full-input entry point: [1, 2048, 32, 128] f32 each -> same shape."""
    _install_ntff_shim()
    from concourse.bass_utils import run_bass_kernel_spmd

    query = np.asarray(query, dtype=np.float32)
    key = np.asarray(key, dtype=np.float32)
    value = np.asarray(value, dtype=np.float32)

    nc = _build()
    in_maps = [_host_prep(query, key, value, c) for c in range(NCORES)]
    res = run_bass_kernel_spmd(
        nc, in_maps, core_ids=list(range(NCORES)),
        trace=_trace, tmpdir=_tmpdir,
    )
    out = np.empty((B, S, H, D), dtype=np.float32)
    for c in range(NCORES):
        o = res.results[c]["o"] / res.results[c]["den"]  # [4, D, S] / [4, 1, S]
        for i in range(HPC):
            out[0, :, c + NCORES * i, :] = o[i].T
    kernel.last_result = res
    return out
